# revision 17
# baseline (speedup 1.0000x reference)
"""GCN 2-layer encoder (gnn_message_passing) on 8 Trainium2 NeuronCores.

Strategy:
  - Nodes sharded 8 ways via a host permutation: dealt round-robin by
    degree (balanced shards), then snake-sorted within each core by
    (total-degree, strict-lo-degree) so each 128-dst window needs a
    near-minimal number of gather planes across all 8 cores.
  - Per layer: each core computes its shard of the gather table
    t = dis * (u @ W) in row form (lhsT = x^T window trick), AllGathers
    the bf16 table, then gathers neighbor rows with dma_gather striped
    across all 4 SWDGE queues (desc-gen runs on Q7 core pair 2q,2q+1 —
    striping parallelizes descriptor generation across all 8 Q7 cores).
  - Aggregation is TRANSPOSED on the PE: each gathered plane (128 rows
    of the table, slot-aligned to the window's 128 dsts) is the
    stationary lhsT; the moving rhs is a per-window diagonal matrix
    carrying dis[dst], so PSUM accumulates aggT[ch, dst] with the full
    symmetric normalization folded in. Self-loop rows come from the
    locally-kept shard table (no gather).
  - Post-processing runs on the idle Activation engine: one
    Prelu(aggT + b) instruction per window (bias and PReLU alpha are
    per-partition in the transposed layout). The layer-0 skip term
    x@Ws + bs is built in PSUM (Ws matmul + a K=1 rank-1 matmul for the
    bias) and folded in with a single DVE add that also casts to bf16,
    yielding u^T directly — no transposes anywhere on device. The final
    output is written column-major and transposed on the host.
"""

import numpy as np

N = 50000
E = 600000
D = 128
P = 128
N_CORES = 8
SHARD = N // N_CORES          # 6250
RANK_ROWS = 6282              # shard rows + 32 zero pad rows
T_ROWS = RANK_ROWS * N_CORES  # 50256
HALF = 32768
HI_BASE = T_ROWS - HALF       # 17488

_CACHE = {}

# SHARD=6250 is not a multiple of 128: 48 full windows + 1 window of 106.
# We pad each shard to 6272 (49*128) dst slots; the last 22 slots of the
# last window are dummy dsts (aggregations computed but discarded).
SHARD_PAD = 6272
WPC = SHARD_PAD // P  # 49


def _row_of(newid):
    return newid + 32 * (newid // SHARD)


def _host_prep(edge_index):
    src = np.asarray(edge_index[0], dtype=np.int64)
    dst = np.asarray(edge_index[1], dtype=np.int64)
    deg = np.bincount(dst, minlength=N).astype(np.int64) + 1  # + self loop
    dis = (1.0 / np.sqrt(deg)).astype(np.float32)

    # deal nodes round-robin by degree to cores (pass 1)
    order = np.argsort(-deg, kind="stable")
    new_id = np.empty(N, dtype=np.int64)
    new_id[order] = np.arange(N)
    pi = (new_id % N_CORES) * SHARD + new_id // N_CORES

    rows = _row_of(np.arange(N, dtype=np.int64))
    ZLO = SHARD                       # row 6250 (rank-0 pad row), < HALF
    ZHI = 6 * RANK_ROWS + SHARD       # row 43942, >= HI_BASE
    assert ZLO < HALF and HI_BASE <= ZHI < T_ROWS

    # Self loops are NOT gathered (each core adds its own shard rows as an
    # extra local plane), so only the real edges enter the streams.
    # Sources with table row in [HI_BASE, HALF) are addressable by BOTH the
    # lo table (rows [0, HALF)) and the hi table (rows [HI_BASE, T_ROWS)).
    def strict_counts(pi_cur):
        s_new = pi_cur[src]
        d_new = pi_cur[dst]
        arow = rows[s_new]
        slo = np.bincount(d_new[arow < HI_BASE], minlength=N)
        shi = np.bincount(d_new[arow >= HALF], minlength=N)
        tot = np.bincount(d_new, minlength=N)
        return slo, shi, tot

    # pass 2: within each core, snake-sort dsts by (tot desc, snake slo)
    # so adjacent windows are homogeneous in both lo and hi plane needs.
    slo_c, shi_c, tot_c = strict_counts(pi)
    final_pos = np.empty(N, dtype=np.int64)
    for c in range(N_CORES):
        ids = np.arange(c * SHARD, (c + 1) * SHARD)
        sl = slo_c[ids]
        tt = tot_c[ids]
        snake_lo = np.where(tt % 2 == 0, sl, -sl)
        key = np.lexsort((-snake_lo, -tt))
        final_pos[ids[key]] = ids
    pi = final_pos[pi]
    inv_pi = np.empty(N, dtype=np.int64)
    inv_pi[pi] = np.arange(N)

    src_new = pi[src]
    dst_new = pi[dst]
    allsrc = src_new
    alldst = dst_new
    srows = rows[allsrc]
    slo_cnt = np.bincount(alldst[srows < HI_BASE], minlength=N)
    shi_cnt = np.bincount(alldst[srows >= HALF], minlength=N)
    tot_cnt = np.bincount(alldst, minlength=N)
    flex_cnt = tot_cnt - slo_cnt - shi_cnt

    def padded(v):
        out = np.zeros((N_CORES, SHARD_PAD), dtype=np.int64)
        out[:, :SHARD] = v.reshape(N_CORES, SHARD)
        return out.reshape(N_CORES, WPC, P)

    slo_w = padded(slo_cnt)
    shi_w = padded(shi_cnt)
    flex_w = padded(flex_cnt)
    tot_w = padded(tot_cnt)
    # Rlo covers strict-lo; flex spills into lo up to Rlo, rest goes hi.
    Rlo = slo_w.max(axis=(0, 2))
    hi_need = tot_w - np.minimum(slo_w + flex_w, Rlo[None, :, None])
    Rhi = np.maximum(hi_need.max(axis=(0, 2)), 0)
    # per-dst lo capacity for the flex assignment below
    lo_cap = Rlo  # [WPC]

    S_lo = int(Rlo.sum()) * P
    S_hi = int(Rhi.sum()) * P
    lo_streams = np.full((N_CORES, S_lo), ZLO, dtype=np.int64)
    hi_streams = np.full((N_CORES, S_hi), ZHI - HI_BASE, dtype=np.int64)

    # category: 0 strict-lo, 1 flex, 2 strict-hi; sort edges by (dst, cat)
    cat = np.where(srows < HI_BASE, 0, np.where(srows < HALF, 1, 2))
    o = np.lexsort((cat, alldst))
    d_sorted = alldst[o]
    s_sorted = srows[o]
    grp_start = np.searchsorted(d_sorted, np.arange(N))
    rank_in_grp = np.arange(len(d_sorted)) - grp_start[d_sorted]
    p_loc = d_sorted % SHARD
    wid = p_loc // P
    part = p_loc % P
    core = d_sorted // SHARD
    # per-dst lo quota
    lo_q_edge = np.minimum(slo_cnt[d_sorted] + flex_cnt[d_sorted], lo_cap[wid])
    to_lo = rank_in_grp < lo_q_edge
    lo_plane_off = np.concatenate([[0], np.cumsum(Rlo)])
    hi_plane_off = np.concatenate([[0], np.cumsum(Rhi)])
    slot_lo = (lo_plane_off[wid] + rank_in_grp) * P + part
    slot_hi = (hi_plane_off[wid] + (rank_in_grp - lo_q_edge)) * P + part
    lo_streams[core[to_lo], slot_lo[to_lo]] = s_sorted[to_lo]
    hi_streams[core[~to_lo], slot_hi[~to_lo]] = s_sorted[~to_lo] - HI_BASE

    def wrap16(vals):
        n = len(vals)
        assert n % 16 == 0
        blk = vals.astype(np.int16).reshape(n // 16, 16).T
        return np.tile(blk, (8, 1)).copy()

    lo_wrapped = np.stack([wrap16(lo_streams[c]) for c in range(N_CORES)])
    hi_wrapped = np.stack([wrap16(hi_streams[c]) for c in range(N_CORES)])

    return dict(
        pi=pi, inv_pi=inv_pi, dis=dis, Rlo=Rlo, Rhi=Rhi,
        lo_wrapped=lo_wrapped, hi_wrapped=hi_wrapped,
        S_lo=S_lo, S_hi=S_hi,
    )


def _build_bass(prep):
    import sys
    if '/opt/trn_rl_repo' not in sys.path:
        sys.path.insert(0, '/opt/trn_rl_repo')
    import concourse.mybir as mybir
    import concourse.tile as tile
    from concourse import bacc
    from collections import defaultdict

    f32 = mybir.dt.float32
    bf16 = mybir.dt.bfloat16
    i16 = mybir.dt.int16
    Prelu = mybir.ActivationFunctionType.Prelu
    Copy = mybir.ActivationFunctionType.Copy

    Rlo, Rhi = prep["Rlo"], prep["Rhi"]
    S_lo, S_hi = prep["S_lo"], prep["S_hi"]

    nc = bacc.Bacc("TRN2", target_bir_lowering=False, debug=False,
                   num_devices=N_CORES, num_swdge_queues=4)

    xT = nc.declare_dram_parameter("xT", [P, SHARD_PAD], f32, isOutput=False)
    dis_col = nc.declare_dram_parameter("dis_col", [P, WPC], f32, isOutput=False)
    dis_diag = nc.declare_dram_parameter("dis_diag", [P, WPC * P], f32, isOutput=False)
    W0p = nc.declare_dram_parameter("W0", [P, D], f32, isOutput=False)
    W1p = nc.declare_dram_parameter("W1", [P, D], f32, isOutput=False)
    Wsp = nc.declare_dram_parameter("Ws", [P, D], f32, isOutput=False)
    b0c = nc.declare_dram_parameter("b0c", [P, 1], f32, isOutput=False)
    b1c = nc.declare_dram_parameter("b1c", [P, 1], f32, isOutput=False)
    bsr = nc.declare_dram_parameter("bsr", [1, D], f32, isOutput=False)
    ac = nc.declare_dram_parameter("ac", [P, 1], f32, isOutput=False)
    lo_idx = nc.declare_dram_parameter("lo_idx", [P, S_lo // 16], i16, isOutput=False)
    hi_idx = nc.declare_dram_parameter("hi_idx", [P, S_hi // 16], i16, isOutput=False)
    yT = nc.declare_dram_parameter("yT", [P, SHARD_PAD], f32, isOutput=True)

    # Gather call schedule: the lo/hi index streams are contiguous across
    # windows, so gather in large fixed-size chunks (C planes each) that
    # ignore window boundaries — far fewer SWDGE calls, each amortizing the
    # per-call Q7 launch/idx-unpack overhead over ~4k rows. Window w's
    # aggregation MMs read planes from whichever chunk tiles hold them.
    CPL = 32  # planes per gather call
    lo_off = np.concatenate([[0], np.cumsum(Rlo)])
    hi_off = np.concatenate([[0], np.cumsum(Rhi)])
    n_lo, n_hi = int(Rlo.sum()), int(Rhi.sum())

    def mk_chunks(n):
        return [(s, min(CPL, n - s)) for s in range(0, n, CPL)]

    lo_chunks = mk_chunks(n_lo)
    hi_chunks = mk_chunks(n_hi)

    with tile.TileContext(nc) as tc:
        with (
            tc.tile_pool(name="const", bufs=1) as cpool,
            tc.tile_pool(name="big", bufs=1) as bigpool,
            tc.tile_pool(name="sbuf", bufs=8) as sbuf,
            tc.tile_pool(name="gw", bufs=5) as gwpool,
            tc.tile_pool(name="psum", bufs=4, space="PSUM") as psum,
            tc.tile_pool(name="psumx", bufs=2, space="PSUM") as psumx,
            tc.tile_pool(name="psum2", bufs=2, space="PSUM") as psum2,
            tc.tile_pool(name="dram", bufs=1, space="DRAM") as dram,
        ):
            def load_cast(dram_t, w, tag):
                tf = sbuf.tile([P, w], f32, tag="ldc")
                nc.sync.dma_start(out=tf[:], in_=dram_t[:])
                tb = cpool.tile([P, w], bf16, tag=tag + "_bf")
                nc.vector.tensor_copy(out=tb[:], in_=tf[:])
                return tb

            def load_f32(dram_t, shape, tag):
                t = cpool.tile(shape, f32, tag=tag + "_f")
                nc.sync.dma_start(out=t[:], in_=dram_t[:])
                return t

            W0t = load_cast(W0p, D, "w0")
            W1t = load_cast(W1p, D, "w1")
            Wst = load_cast(Wsp, D, "ws")
            b0t = load_f32(b0c, [P, 1], "b0")
            b1t = load_f32(b1c, [P, 1], "b1")
            at = load_f32(ac, [P, 1], "a")
            disC = load_f32(dis_col, [P, WPC], "disc")

            bs_f = cpool.tile([1, D], f32)
            nc.sync.dma_start(out=bs_f[:], in_=bsr[:])
            bs_b = cpool.tile([1, D], bf16)
            nc.vector.tensor_copy(out=bs_b[:], in_=bs_f[:])
            ones_b = cpool.tile([1, D], bf16)
            nc.vector.memset(ones_b[:], 1.0)

            # one shared f32 staging tile, reused for both big input casts
            stage_f = bigpool.tile([P, SHARD_PAD], f32)
            ddiag = bigpool.tile([P, WPC * P], bf16)
            nc.sync.dma_start(out=stage_f[:], in_=dis_diag[:])
            nc.vector.tensor_copy(out=ddiag[:], in_=stage_f[:])

            xT_t = bigpool.tile([P, SHARD_PAD], bf16)
            nc.sync.dma_start(out=stage_f[:], in_=xT[:])
            nc.vector.tensor_copy(out=xT_t[:], in_=stage_f[:])

            lo_t = bigpool.tile([P, S_lo // 16], i16)
            nc.sync.dma_start(out=lo_t[:], in_=lo_idx[:])
            hi_t = bigpool.tile([P, S_hi // 16], i16)
            nc.sync.dma_start(out=hi_t[:], in_=hi_idx[:])

            tin = [dram.tile([RANK_ROWS, D], bf16, tag=f"tin{l}", name=f"tin{l}") for l in range(2)]
            tfull = [dram.tile([T_ROWS, D], bf16, tag=f"tfull{l}", name=f"tfull{l}", addr_space="Shared") for l in range(2)]
            zpad = cpool.tile([32, D], bf16)
            nc.vector.memzero(zpad[:])
            for l in range(2):
                nc.sync.dma_start(out=tin[l][SHARD:RANK_ROWS, :], in_=zpad[:])

            loc0 = bigpool.tile([P, WPC, D], bf16)
            loc1 = bigpool.tile([P, WPC, D], bf16)
            local_t = [loc0, loc1]
            u1T = bigpool.tile([P, SHARD_PAD], bf16)

            def build_win(layer, src_T, w):
                # table rows t[n, o] = dis[n] * (u @ W)[n, o] for window w
                Wt = W0t if layer == 0 else W1t
                pt = psum2.tile([P, P], f32, tag="pp")
                nc.tensor.matmul(out=pt[:], lhsT=src_T[:, w * P:(w + 1) * P],
                                 rhs=Wt[:], start=True, stop=True)
                nc.scalar.activation(local_t[layer][:, w, :], pt[:], Copy,
                                     scale=disC[:, w:w + 1])
                lim = min(SHARD - w * P, P)
                nc.sync.dma_start(out=tin[layer][w * P:w * P + lim, :],
                                  in_=local_t[layer][:lim, w, :])

            def all_gather(layer):
                nc.gpsimd.collective_compute(
                    "AllGather", mybir.AluOpType.bypass,
                    replica_groups=[list(range(N_CORES))],
                    ins=[tin[layer].opt()], outs=[tfull[layer].opt()],
                )

            qrr = [0]
            nidx_regs = {}

            def gather_chunk(T, sid, s, k, tiles):
                idx_t = lo_t if sid == 0 else hi_t
                tbl_ap = T[0:HALF, :] if sid == 0 else T[HI_BASE:T_ROWS, :]
                nidx = k * P
                if nidx not in nidx_regs:
                    nidx_regs[nidx] = nc.gpsimd.to_reg(nidx)
                gt = gwpool.tile([P, k, D], bf16, tag=f"gw{sid}")
                nc.gpsimd.dma_gather(
                    out_ap=gt[:],
                    in_ap=tbl_ap,
                    idxs_ap=idx_t[:, s * 8:(s + k) * 8],
                    num_idxs=nidx, num_idxs_reg=nidx_regs[nidx],
                    elem_size=D,
                    single_packet=False,
                    queue_num=qrr[0] % 4,
                )
                qrr[0] += 1
                tiles[sid][s // CPL] = gt

            PREFETCH = 2 * CPL  # planes of gather lookahead per stream

            def run_layer(layer, post_fn):
                T = tfull[layer]
                tiles = ([None] * len(lo_chunks), [None] * len(hi_chunks))
                iters = [iter(lo_chunks), iter(hi_chunks)]
                totals = [n_lo, n_hi]
                issued = [0, 0]  # planes issued per sid

                def ensure(sid, upto):
                    upto = min(upto, totals[sid])
                    while issued[sid] < upto:
                        s, k = next(iters[sid])
                        gather_chunk(T, sid, s, k, tiles)
                        issued[sid] = s + k

                def plane_ap(sid, p):
                    gt = tiles[sid][p // CPL]
                    return gt[:, p % CPL, :]

                for w in range(WPC):
                    rlo, rhi = int(Rlo[w]), int(Rhi[w])
                    ensure(0, int(lo_off[w]) + rlo + PREFETCH)
                    ensure(1, int(hi_off[w]) + rhi + PREFETCH)
                    # transposed aggregation: aggT[ch, dst] accumulates
                    # plane^T @ diag(dis[dst]) over gather planes + self loop
                    dg = ddiag[:, w * P:(w + 1) * P]
                    agg = psum.tile([P, P], f32, tag="agg")
                    first = True
                    for p in range(int(lo_off[w]), int(lo_off[w]) + rlo):
                        nc.tensor.matmul(out=agg[:], lhsT=plane_ap(0, p),
                                         rhs=dg, start=first, stop=False)
                        first = False
                    for p in range(int(hi_off[w]), int(hi_off[w]) + rhi):
                        nc.tensor.matmul(out=agg[:], lhsT=plane_ap(1, p),
                                         rhs=dg, start=first, stop=False)
                        first = False
                    nc.tensor.matmul(out=agg[:], lhsT=local_t[layer][:, w, :],
                                     rhs=dg, start=first, stop=True)
                    post_fn(w, agg)

            def post0(w, agg):
                # skip term (x @ Ws + bs)^T for this window, built in PSUM
                xws = psumx.tile([P, P], f32, tag="xws")
                nc.tensor.matmul(out=xws[:], lhsT=Wst[:],
                                 rhs=xT_t[:, w * P:(w + 1) * P],
                                 start=True, stop=False)
                nc.tensor.matmul(out=xws[:], lhsT=bs_b[:], rhs=ones_b[:],
                                 start=False, stop=True)
                h = sbuf.tile([P, P], f32, tag="h")
                nc.scalar.activation(h[:], agg[:], Prelu, bias=b0t[:, 0:1],
                                     alpha=at[:, 0:1])
                nc.vector.tensor_add(u1T[:, w * P:(w + 1) * P], h[:], xws[:])
                build_win(1, u1T, w)

            def post1(w, agg):
                y = sbuf.tile([P, P], f32, tag="y")
                nc.scalar.activation(y[:], agg[:], Prelu, bias=b1t[:, 0:1],
                                     alpha=at[:, 0:1])
                nc.sync.dma_start(out=yT[:, w * P:(w + 1) * P], in_=y[:])

            for w in range(WPC):
                build_win(0, xT_t, w)
            all_gather(0)
            run_layer(0, post0)
            all_gather(1)
            run_layer(1, post1)

    nc.compile()
    return nc


def kernel(**inputs):
    import sys
    if '/opt/trn_rl_repo' not in sys.path:
        sys.path.insert(0, '/opt/trn_rl_repo')
    from concourse.bass_utils import run_bass_kernel_spmd

    x = np.asarray(inputs["x"], dtype=np.float32)
    edge_index = np.asarray(inputs["edge_index"])
    W0 = np.asarray(inputs["W0"], dtype=np.float32)
    b0 = np.asarray(inputs["b0"], dtype=np.float32)
    W1 = np.asarray(inputs["W1"], dtype=np.float32)
    b1 = np.asarray(inputs["b1"], dtype=np.float32)
    Ws = np.asarray(inputs["Ws"], dtype=np.float32)
    bs = np.asarray(inputs["bs"], dtype=np.float32)
    a = np.asarray(inputs["a"], dtype=np.float32)

    if "prep" not in _CACHE:
        _CACHE["prep"] = _host_prep(edge_index)
        _CACHE["nc"] = _build_bass(_CACHE["prep"])
    prep = _CACHE["prep"]
    nc = _CACHE["nc"]

    pi, inv_pi, dis = prep["pi"], prep["inv_pi"], prep["dis"]
    x_perm = x[inv_pi]
    dis_perm = dis[inv_pi]

    in_maps = []
    for c in range(N_CORES):
        sl = slice(c * SHARD, (c + 1) * SHARD)
        xs = np.zeros((SHARD_PAD, D), dtype=np.float32)
        xs[:SHARD] = x_perm[sl]
        ds = np.zeros(SHARD_PAD, dtype=np.float32)
        ds[:SHARD] = dis_perm[sl]
        # per-window diagonal of dis[dst]: [lane, w*P + dst] nonzero at dst==lane
        ddg = np.zeros((P, WPC, P), dtype=np.float32)
        lanes = np.arange(P)
        for w in range(WPC):
            ddg[lanes, w, lanes] = ds[w * P + lanes]
        in_maps.append({
            "xT": np.ascontiguousarray(xs.T),
            "dis_col": np.ascontiguousarray(ds.reshape(WPC, P).T),
            "dis_diag": ddg.reshape(P, WPC * P),
            "W0": W0, "W1": W1, "Ws": Ws,
            "b0c": b0.reshape(P, 1),
            "b1c": b1.reshape(P, 1),
            "bsr": bs.reshape(1, D),
            "ac": a.reshape(P, 1),
            "lo_idx": prep["lo_wrapped"][c],
            "hi_idx": prep["hi_wrapped"][c],
        })

    kwargs = _CACHE.get("run_kwargs", {})
    res = run_bass_kernel_spmd(nc, in_maps, core_ids=list(range(N_CORES)),
                               **kwargs)
    out_perm = np.concatenate(
        [res.results[c]["yT"].T[:SHARD] for c in range(N_CORES)], axis=0)
    out = out_perm[pi]
    _CACHE["last_res"] = res
    return out.astype(np.float32)
